# revision 1
# baseline (speedup 1.0000x reference)
"""NeighborDiscriminator kernel for 8x Trainium2 NeuronCores.

Math (reference): augmented-L2 kNN search, k=10, over n=100000 database rows,
B=1024 queries, d=512, followed by max over the k neighbors of
act_i = w_i - ||x_i - q||.

Selection key per (query q, candidate i):
    d2aug = ||q||^2 - 2 q.x_i + ||x_i||^2 + (max(w) - w_i)
Per-row-constant terms (||q||^2, max(w)) don't change the per-query ordering,
so the device ranks by  A = 2 q.x_i + (w_i - ||x_i||^2)  (descending A ==
ascending d2aug).

Distribution: X / w sharded over 8 cores along n (12500 rows each), queries
replicated.  Each core computes A = [1024, 12500] via fp32r matmuls (the
(w - ||x||^2) term enters PSUM through a rank-1 ones x aug matmul), and
reduces each 500-wide candidate tile to its per-query top-8 (DVE max8 +
max_index).  Host merges the 8 x 200 chunk-candidates per query, takes the
top-32 by device score, exactly re-ranks those in float64 (device fp32r noise
is ~0.15 abs vs ~0.7 typical rank gaps, so 32 is a huge safety margin),
keeps the k nearest, and returns max_k(w - dist).

A global top-k member can only be missed if >8 of the global top-10 land in
one 500-wide chunk (P ~ 1e-17) — the per-chunk top-8 is exhaustive in
practice.
"""

import numpy as np

import concourse.bacc as bacc
import concourse.mybir as mybir
from concourse.tile import TileContext
from concourse.bass_utils import run_bass_kernel_spmd

B = 1024            # queries
N_TOTAL = 100000    # database rows
D = 512             # feature dim
M = 8               # cores
NS = N_TOTAL // M   # 12500 rows per core
CT = 500            # candidate tile width (PSUM bank = 512 fp32)
NT = NS // CT       # 25 candidate tiles per core
QT = B // 128       # 8 query tiles
KC = D // 128       # 4 contraction chunks
TOP = 8             # top-8 per chunk (DVE max8)
NOUT = NT * TOP     # 200 candidates per query per core

_cached_nc = None


def _build():
    nc = bacc.Bacc(
        "TRN2",
        target_bir_lowering=False,
        debug=False,
        enable_asserts=False,
        num_devices=M,
    )
    f32r = mybir.dt.float32r
    f32 = mybir.dt.float32
    q2T = nc.dram_tensor("q2T", [D, B], f32r, kind="ExternalInput")
    xT = nc.dram_tensor("xT", [D, NS], f32r, kind="ExternalInput")
    # aug = [w - ||x||^2 (NS) | ones (128)] in one row: a single DMA so the
    # rank-1 matmul carries a single semaphore wait (Matmult wait-slot limit).
    aug = nc.dram_tensor("aug", [1, NS + 128], f32r, kind="ExternalInput")
    vals = nc.dram_tensor("vals", [B, NOUT], f32, kind="ExternalOutput")
    idxs = nc.dram_tensor("idxs", [B, NOUT], mybir.dt.uint16, kind="ExternalOutput")

    with TileContext(nc) as tc:
        with (
            tc.tile_pool(name="const", bufs=1) as cpool,
            tc.tile_pool(name="xs", bufs=3) as xpool,
            tc.tile_pool(name="a", bufs=4) as apool,
            tc.tile_pool(name="out", bufs=1) as opool,
            tc.tile_pool(name="ps", bufs=8, space="PSUM") as pspool,
        ):
            q_tile = cpool.tile([128, KC, B], f32r)
            nc.sync.dma_start(out=q_tile, in_=q2T.rearrange("(c p) m -> p c m", p=128))
            aug_tile = cpool.tile([1, NS + 128], f32r)
            nc.sync.dma_start(out=aug_tile, in_=aug[:, :])
            ones = aug_tile[:, NS:]

            vals_sb = opool.tile([128, QT * NOUT], f32)
            idxs_sb = opool.tile([128, QT * NOUT], mybir.dt.uint16)

            xT_r = xT.rearrange("(c p) n -> p c n", p=128)
            for t in range(NT):
                x_tile = xpool.tile([128, KC, CT], f32r)
                nc.sync.dma_start(out=x_tile, in_=xT_r[:, :, t * CT : (t + 1) * CT])
                for q in range(QT):
                    ps = pspool.tile([128, CT], f32)
                    for c in range(KC):
                        nc.tensor.matmul(
                            ps,
                            lhsT=q_tile[:, c, q * 128 : (q + 1) * 128],
                            rhs=x_tile[:, c, :],
                            start=(c == 0),
                            stop=False,
                        )
                    nc.tensor.matmul(
                        ps,
                        lhsT=ones,
                        rhs=aug_tile[:, t * CT : (t + 1) * CT],
                        start=False,
                        stop=True,
                    )
                    a_tile = apool.tile([128, CT], f32)
                    nc.scalar.copy(a_tile, ps)
                    o = q * NOUT + t * TOP
                    nc.vector.max(out=vals_sb[:, o : o + TOP], in_=a_tile)
                    nc.vector.max_index(
                        out=idxs_sb[:, o : o + TOP],
                        in_max=vals_sb[:, o : o + TOP],
                        in_values=a_tile,
                    )

            nc.sync.dma_start(
                out=vals.rearrange("(q p) k -> p q k", p=128),
                in_=vals_sb.rearrange("p (q k) -> p q k", q=QT),
            )
            nc.sync.dma_start(
                out=idxs.rearrange("(q p) k -> p q k", p=128),
                in_=idxs_sb.rearrange("p (q k) -> p q k", q=QT),
            )
    nc.compile()
    return nc


def _get_nc():
    global _cached_nc
    if _cached_nc is None:
        _cached_nc = _build()
    return _cached_nc


def _device_candidates(X_tilde, X, w):
    """Run the SPMD search. Returns per-query merged (device_score, global_idx)
    arrays of shape [B, M * NOUT]."""
    q2T = np.ascontiguousarray((2.0 * X_tilde).T).astype(np.float32)
    x_sq = np.einsum("nd,nd->n", X.astype(np.float64), X.astype(np.float64))
    wmx = (w[:, 0].astype(np.float64) - x_sq).astype(np.float32)

    in_maps = []
    for c in range(M):
        sl = slice(c * NS, (c + 1) * NS)
        in_maps.append(
            {
                "q2T": q2T,
                "xT": np.ascontiguousarray(X[sl].T),
                "aug": np.concatenate([wmx[sl], np.ones(128, np.float32)])[None, :],
            }
        )

    res = run_bass_kernel_spmd(_get_nc(), in_maps, core_ids=list(range(M)))
    vals = np.stack([res.results[c]["vals"] for c in range(M)], axis=1)  # [B, M, 200]
    idxs = np.stack([res.results[c]["idxs"] for c in range(M)], axis=1)
    # local chunk index -> global row: core*NS + chunk*CT + idx
    chunk = (np.arange(NOUT, dtype=np.int64) // TOP) * CT  # [200]
    gidx = (
        np.arange(M, dtype=np.int64)[None, :, None] * NS
        + chunk[None, None, :]
        + idxs.astype(np.int64)
    )
    return vals.reshape(B, M * NOUT), gidx.reshape(B, M * NOUT)


def kernel(X_tilde, X, w, k):
    k = int(k)
    assert k <= 16, f"per-chunk top-8 merge assumes small k, got {k}"
    X_tilde = np.asarray(X_tilde, dtype=np.float32)
    X = np.asarray(X, dtype=np.float32)
    w = np.asarray(w, dtype=np.float32).reshape(N_TOTAL, 1)

    dev_scores, gidx = _device_candidates(X_tilde, X, w)

    # Top-32 by device score, then exact float64 re-rank of those candidates.
    margin = max(32, 2 * k)
    part = np.argpartition(-dev_scores, margin, axis=1)[:, :margin]
    cand = np.take_along_axis(gidx, part, axis=1)  # [B, margin] global rows

    Xc = X[cand].astype(np.float64)                  # [B, margin, d]
    diff = Xc - X_tilde[:, None, :].astype(np.float64)
    d2 = np.einsum("bkd,bkd->bk", diff, diff)        # exact squared distances
    wc = w[cand, 0].astype(np.float64)               # [B, margin]
    key = d2 - wc                                    # ascending == d2aug order
    sel = np.argpartition(key, k, axis=1)[:, :k]     # exact k nearest
    d2k = np.take_along_axis(d2, sel, axis=1)
    wk = np.take_along_axis(wc, sel, axis=1)
    act = wk - np.sqrt(d2k)                          # K_COEF = 1.0
    return act.max(axis=1).astype(np.float32)

